# revision 32
# baseline (speedup 1.0000x reference)
"""Trainium2 Bass kernel for the AdjacencyPolicyModule MLP.

Strategy (data-parallel over batch, 8 NeuronCores, 512 graphs/core):
  - Host: scatter edge lists into a dense transposed adjacency laid out as
    [tile, dst, src*128+g] in fp8e4 (values 0/1 are exact; bytes written
    directly, no float conversion). Weights converted to bf16 and
    pre-arranged into the chunked SBUF layouts the matmuls want.
  - Device pass 1, software-pipelined over 128-graph tiles:
      step t:   emb0(t)  = adjT(t) x W0   (128 fp8xbf16 matmuls + bias row)
      step t+1: epiA(t)  = LN+ReLU -> h0T transposes -> W1 matmuls
      step t+2: epiB(t)  = LN+ReLU -> h1T transposes -> Wm matmuls -> stash
    so the PE chews tile t+1's big matmul group while tile t's LayerNorm
    chain runs on DVE/ACT.
  - Device pass 2, same 2-stage pipeline:
      A(t): embT transposes, We0/Wx0 matmuls (launches LN chains)
      B(t): heT/hxT transposes, exit head, 32 edge-head column matmuls,
            stage copies, output DMA in 4 column groups.
  - Output staged in bf16; host upcasts to f32 and concatenates.
"""

import os

import numpy as np
import ml_dtypes

import concourse.bass as bass
import concourse.tile as tile
from concourse import bacc, mybir
from concourse.bass_utils import run_bass_kernel_spmd
from concourse.masks import make_identity
from concourse.tile_rust import add_dep_helper

B, N, D = 4096, 128, 256
EO = N * N - N          # 16256 edge-action outputs
OUTW = EO + 1           # 16257 with exit action
NCORES = 8
GPC = B // NCORES       # 512 graphs per core
TPC = GPC // 128        # 4 row tiles per core
COLT = 508              # edge-head column tile (32 * 508 = 16256)
NCOL = EO // COLT

BF = ml_dtypes.bfloat16
F8 = ml_dtypes.float8_e4m3

_cache = {}

LAST_EXEC_NS = None


class Ctx:
    """Shared handles for the kernel builder."""
    pass


def _ln_relu(nc, cx, x_ps, gi, bi, out_bf):
    """out_bf = relu(layernorm(x_ps) * g + be), f32 PSUM in, bf16 SBUF out."""
    acts = cx.acts
    stats = acts.tile([128, 6], mybir.dt.float32, tag="stats", bufs=2)
    nc.vector.bn_stats(out=stats[:], in_=x_ps[:])
    mv = acts.tile([128, 2], mybir.dt.float32, tag="mv", bufs=2)
    nc.vector.bn_aggr(out=mv[:], in_=stats[:])
    rstd = acts.tile([128, 1], mybir.dt.float32, tag="rstd", bufs=2)
    nc.scalar.activation(
        out=rstd[:], in_=mv[:, 1:2],
        func=mybir.ActivationFunctionType.Sqrt,
        bias=cx.eps[:, 0:1], scale=1.0,
    )
    nc.vector.reciprocal(out=rstd[:], in_=rstd[:])
    xn = acts.tile([128, D], mybir.dt.float32, tag="xn", bufs=3)
    nc.vector.tensor_scalar(
        out=xn[:], in0=x_ps[:],
        scalar1=mv[:, 0:1], scalar2=rstd[:, 0:1],
        op0=mybir.AluOpType.subtract, op1=mybir.AluOpType.mult,
    )
    nc.vector.tensor_tensor(
        out=xn[:], in0=xn[:], in1=cx.lnp[:, gi * D:(gi + 1) * D],
        op=mybir.AluOpType.mult,
    )
    nc.vector.tensor_tensor(
        out=xn[:], in0=xn[:], in1=cx.lnp[:, bi * D:(bi + 1) * D],
        op=mybir.AluOpType.add,
    )
    nc.scalar.activation(
        out=out_bf[:], in_=xn[:], func=mybir.ActivationFunctionType.Relu,
    )


def _transpose(nc, cx, src_bf):
    """src_bf [128(g), 256(d)] bf16 -> [128(d%128), k*128+g] bf16 (2 chunks)."""
    dst = cx.acts.tile([128, D], mybir.dt.bfloat16, tag="hT", bufs=3)
    for k in range(2):
        pt = cx.psT.tile([128, 128], mybir.dt.bfloat16, tag="tr")
        nc.tensor.transpose(
            out=pt[:], in_=src_bf[:, k * 128:(k + 1) * 128], identity=cx.ident[:],
        )
        nc.vector.tensor_copy(out=dst[:, k * 128:(k + 1) * 128], in_=pt[:])
    return dst


def _dense_small(nc, cx, pool, tag, lhsT, widx, boff):
    """[128,256] = lhsT.T @ W_widx + bias; returns PSUM tile."""
    ps = pool.tile([128, D], mybir.dt.float32, tag=tag)
    for k in range(2):
        nc.tensor.matmul(
            ps[:], lhsT=lhsT[:, k * 128:(k + 1) * 128],
            rhs=cx.wsm[:, widx * 2 * D + k * D: widx * 2 * D + (k + 1) * D],
            start=(k == 0), stop=False,
        )
    nc.tensor.matmul(
        ps[:], lhsT=cx.ones[0:1, :], rhs=cx.brow[0:1, boff:boff + D],
        start=False, stop=True,
    )
    return ps


def _build():
    nc = bacc.Bacc("TRN2", target_bir_lowering=False, debug=False,
                   num_devices=NCORES)
    dt = mybir.dt

    adjT_d = nc.declare_dram_parameter("adjT", [TPC, 128, N * 128], dt.float8e4, isOutput=False)
    w0_d = nc.declare_dram_parameter("w0", [128, 128 * D], dt.bfloat16, isOutput=False)
    we1_d = nc.declare_dram_parameter("we1", [128, 2 * EO], dt.bfloat16, isOutput=False)
    wsm_d = nc.declare_dram_parameter("wsm", [4, 128, 2 * D], dt.bfloat16, isOutput=False)
    wx1_d = nc.declare_dram_parameter("wx1", [128, 2], dt.bfloat16, isOutput=False)
    brow_d = nc.declare_dram_parameter("brow", [1, 5 * D + 1], dt.bfloat16, isOutput=False)
    lnp_d = nc.declare_dram_parameter("lnp", [128, 8 * D], dt.float32, isOutput=False)
    out_d = nc.declare_dram_parameter("out", [GPC, OUTW], dt.bfloat16, isOutput=True)

    cx = Ctx()

    with tile.TileContext(nc) as tc:
        with (
            tc.tile_pool(name="persist", bufs=1) as persist,
            tc.tile_pool(name="acts", bufs=3) as acts,
        ):
            cx.acts = acts

            # ---- pass 1 (with critical-path-first DMA ordering) ----
            with tc.tile_pool(name="w0p", bufs=1) as w0p, \
                 tc.tile_pool(name="adjp", bufs=4) as adjp, \
                 tc.tile_pool(name="psE", bufs=2, space="PSUM") as psE, \
                 tc.tile_pool(name="psH", bufs=2, space="PSUM") as psH, \
                 tc.tile_pool(name="psT", bufs=2, space="PSUM") as psT, \
                 tc.tile_pool(name="psM", bufs=2, space="PSUM") as psM:
                cx.psT = psT

                # adjacency tile 0 first half, then W0 slices interleaved so
                # the very first matmuls can start ~6us in.
                def adj_half(t, h):
                    ah = adjp.tile([128, 64 * 128], dt.float8e4, tag="adj")
                    nc.sync.dma_start(
                        out=ah[:],
                        in_=adjT_d[t, :, h * 64 * 128:(h + 1) * 64 * 128])
                    return ah

                adj_q = [adj_half(0, 0)]
                w0t = []
                for i in range(8):
                    w = w0p.tile([128, 16 * D], dt.bfloat16, tag=f"w0_{i}")
                    nc.sync.dma_start(
                        out=w[:], in_=w0_d[:, i * 16 * D:(i + 1) * 16 * D])
                    w0t.append(w)
                    if i == 1:
                        adj_q.append(adj_half(0, 1))
                adj_q.append(adj_half(1, 0))
                adj_q.append(adj_half(1, 1))

                # small persistent constants (gpsimd queue, off critical path)
                wsm = persist.tile([128, 4 * 2 * D], dt.bfloat16)
                for i in range(4):
                    nc.gpsimd.dma_start(
                        out=wsm[:, i * 2 * D:(i + 1) * 2 * D], in_=wsm_d[i])
                wx1 = persist.tile([128, 2], dt.bfloat16)
                nc.gpsimd.dma_start(out=wx1[:], in_=wx1_d[:])
                brow = persist.tile([1, 5 * D + 1], dt.bfloat16)
                nc.gpsimd.dma_start(out=brow[:], in_=brow_d[:])
                lnp = persist.tile([128, 8 * D], dt.float32)
                nc.gpsimd.dma_start(out=lnp[:], in_=lnp_d[:])
                ident = persist.tile([128, 128], dt.bfloat16)
                make_identity(nc, ident[:])
                ones = persist.tile([1, 128], dt.bfloat16)
                nc.vector.memset(ones[:], 1.0)
                eps = persist.tile([128, 1], dt.float32)
                nc.vector.memset(eps[:], 1e-5)
                emb_stash = persist.tile([128, TPC * D], dt.bfloat16)
                we1 = persist.tile([128, 2 * EO], dt.bfloat16)

                cx.wsm, cx.wx1, cx.brow, cx.lnp = wsm, wx1, brow, lnp
                cx.ident, cx.ones, cx.eps = ident, ones, eps
                cx.emb_stash, cx.we1 = emb_stash, we1

                # pipeline state
                emb0_ps = [None] * TPC
                emb0_last = [None] * TPC
                h0_bf = [None] * TPC
                h1_ps = [None] * TPC
                h1_bf = [None] * TPC

                def stage_emb0(t):
                    adj = adj_q[:2]
                    del adj_q[:2]
                    ps = psE.tile([128, D], dt.float32, tag="e0")
                    for s in range(128):
                        nc.tensor.matmul(
                            ps[:],
                            lhsT=adj[s // 64][:, (s % 64) * 128:(s % 64 + 1) * 128],
                            rhs=w0t[s // 16][:, (s % 16) * D:(s % 16 + 1) * D],
                            start=(s == 0), stop=False,
                        )
                    emb0_last[t] = nc.tensor.matmul(
                        ps[:], lhsT=ones[0:1, :], rhs=brow[0:1, 0:D],
                        start=False, stop=True,
                    )
                    emb0_ps[t] = ps
                    # prefetch the next-next tile's adjacency
                    if t + 2 < TPC:
                        adj_q.append(adj_half(t + 2, 0))
                        adj_q.append(adj_half(t + 2, 1))

                def stage_epiA(t):
                    h0 = acts.tile([128, D], dt.bfloat16, tag="h", bufs=4)
                    _ln_relu(nc, cx, emb0_ps[t], 0, 1, h0)
                    h0_bf[t] = h0
                    h0T = _transpose(nc, cx, h0)
                    h1_ps[t] = _dense_small(nc, cx, psH, "h1", h0T, 0, D)

                def stage_epiB(t):
                    h1 = acts.tile([128, D], dt.bfloat16, tag="h", bufs=4)
                    _ln_relu(nc, cx, h1_ps[t], 2, 3, h1)
                    h1_bf[t] = h1
                    h1T = _transpose(nc, cx, h1)
                    # embT computed directly: embT[d2,g] = sum_d1 Wm[d1,d2]
                    # h1T[d1,g] + bm[d2]; avoids a later transpose on the
                    # A-stage critical path.
                    for h in range(2):
                        eps_h = psM.tile([128, 128], dt.float32, tag="em")
                        for k in range(2):
                            nc.tensor.matmul(
                                eps_h[:],
                                lhsT=wsm[:, 2 * D + k * D + h * 128:
                                         2 * D + k * D + (h + 1) * 128],
                                rhs=h1T[:, k * 128:(k + 1) * 128],
                                start=(k == 0), stop=False)
                        nc.tensor.matmul(
                            eps_h[:],
                            lhsT=brow[0:1, 2 * D + h * 128:2 * D + (h + 1) * 128],
                            rhs=ones[0:1, :], start=False, stop=True)
                        nc.vector.tensor_copy(
                            out=emb_stash[:, t * D + h * 128:t * D + (h + 1) * 128],
                            in_=eps_h[:])

                heT_t = [None] * TPC
                hxT_t = [None] * TPC

                def stage_A(t, pool, tag):
                    embT = emb_stash[:, t * D:(t + 1) * D]
                    he_ps = _dense_small(nc, cx, pool, tag, embT, 2, 3 * D)
                    hx_ps = _dense_small(nc, cx, pool, tag, embT, 3, 4 * D)
                    he = acts.tile([128, D], dt.bfloat16, tag="ho", bufs=6)
                    _ln_relu(nc, cx, he_ps, 4, 5, he)
                    hx = acts.tile([128, D], dt.bfloat16, tag="ho", bufs=6)
                    _ln_relu(nc, cx, hx_ps, 6, 7, hx)
                    heT_t[t] = he
                    hxT_t[t] = hx

                for step in range(TPC + 2):
                    if step < TPC:
                        stage_emb0(step)
                    if 1 <= step < TPC + 1:
                        stage_epiA(step - 1)
                    if step >= 2:
                        stage_epiB(step - 2)
                    # head LN chains for tiles 0/1 emitted after the next
                    # tile's layer-0 group (their embT input is ready by then,
                    # so the PE never waits; LN chains overlap the tail)
                    if step in (3, 4):
                        stage_A(step - 3, psM, "em")

                # we1 trickles in behind the critical path: each slice DMA is
                # anchored to a layer-0 matmul group so it does not compete
                # with adjT/w0 for HBM bandwidth at startup.
                W14 = EO // 2  # 8128 = 16 col groups per slice
                anchors = [1, 1, 2, 2]
                for i in range(4):
                    dma = nc.gpsimd.dma_start(
                        out=we1[:, i * W14:(i + 1) * W14],
                        in_=we1_d[:, i * W14:(i + 1) * W14])
                    add_dep_helper(
                        dma.ins, emb0_last[anchors[i]].ins,
                        reason="delay we1 load behind layer-0")

            # ---- pass 2: heads ----
            with tc.tile_pool(name="stage", bufs=2) as stagep, \
                 tc.tile_pool(name="psHead", bufs=2, space="PSUM") as psHead, \
                 tc.tile_pool(name="psT2", bufs=2, space="PSUM") as psT2, \
                 tc.tile_pool(name="psBig", bufs=4, space="PSUM") as psBig:
                cx.psT = psT2

                def stage_B(t):
                    heT = _transpose(nc, cx, heT_t[t])
                    hxT = _transpose(nc, cx, hxT_t[t])
                    stage = stagep.tile([128, OUTW], dt.bfloat16)

                    # exit head (bias bx1 applied host-side)
                    exps = psBig.tile([128, COLT], dt.float32, tag="big")
                    for k in range(2):
                        nc.tensor.matmul(
                            exps[:, 0:1], lhsT=hxT[:, k * 128:(k + 1) * 128],
                            rhs=wx1[:, k:k + 1], start=(k == 0), stop=(k == 1))
                    nc.vector.tensor_copy(
                        out=stage[:, EO:EO + 1], in_=exps[:, 0:1])

                    # edge head (bias beb1 applied host-side); we1 is stored
                    # k-interleaved per column group. Output DMA'd in column
                    # groups, with a small final group to shorten the tail.
                    dma_at = {7: (0, 8), 15: (8, 16), 23: (16, 24),
                              29: (24, 30), 31: (30, 32)}
                    for c in range(NCOL):
                        pb = psBig.tile([128, COLT], dt.float32, tag="big")
                        for k in range(2):
                            nc.tensor.matmul(
                                pb[:],
                                lhsT=heT[:, k * 128:(k + 1) * 128],
                                rhs=we1[:, (2 * c + k) * COLT:(2 * c + k + 1) * COLT],
                                start=(k == 0), stop=(k == 1))
                        if c % 2 == 0:
                            nc.vector.tensor_copy(
                                out=stage[:, c * COLT:(c + 1) * COLT], in_=pb[:])
                        else:
                            nc.scalar.activation(
                                out=stage[:, c * COLT:(c + 1) * COLT], in_=pb[:],
                                func=mybir.ActivationFunctionType.Copy)
                        if c in dma_at:
                            glo, ghi = dma_at[c]
                            lo = glo * COLT
                            hi = ghi * COLT if ghi < NCOL else OUTW
                            nc.sync.dma_start(
                                out=out_d[t * 128:(t + 1) * 128, lo:hi],
                                in_=stage[:, lo:hi])

                sched = [("B", 0), ("A", 2), ("B", 1), ("A", 3), ("B", 2),
                         ("B", 3)]
                for kind, t in sched:
                    if kind == "A":
                        stage_A(t, psHead, "head")
                    else:
                        stage_B(t)

    nc.compile()
    return nc


def _prep(edge_src, edge_dst, edge_batch,
          W0, b0, g0, be0, W1, b1, g1, be1, Wm, bm,
          We0, beb0, ge0, bee0, We1, beb1,
          Wx0, bx0, gx0, bxe0, Wx1, bx1):
    # adjacency: [32 tiles, dst, src, g], fp8 bytes written directly
    A = np.zeros((B, N, N), np.uint8)
    A[np.asarray(edge_batch), np.asarray(edge_src), np.asarray(edge_dst)] = 0x38
    adjT = np.ascontiguousarray(
        A.reshape(B // 128, 128, N, N).transpose(0, 3, 2, 1)
    ).reshape(B // 128, 128, N * 128).view(F8)

    def chunked(W, dout):
        # [K, dout] -> [128, (K//128)*dout] with [p, k*dout+j] = W[k*128+p, j]
        K = W.shape[0]
        return np.ascontiguousarray(
            np.asarray(W, np.float32).reshape(K // 128, 128, dout)
            .transpose(1, 0, 2)).astype(BF).reshape(128, (K // 128) * dout)

    w0 = chunked(W0, D)
    # We1 k-interleaved per column group: [p, (c*2+k)*COLT+j] = We1[k*128+p, c*COLT+j]
    we1 = np.ascontiguousarray(
        np.asarray(We1, np.float32).reshape(2, 128, NCOL, COLT)
        .transpose(1, 2, 0, 3)).astype(BF).reshape(128, 2 * EO)
    wsm = np.stack([chunked(W, D) for W in (W1, Wm, We0, Wx0)])
    wx1 = chunked(Wx1, 1)
    brow = np.concatenate(
        [np.asarray(v, np.float32).ravel() for v in (b0, b1, bm, beb0, bx0, bx1)]
    )[None, :].astype(BF)
    lnp = np.concatenate(
        [np.tile(np.asarray(p, np.float32)[None, :], (128, 1))
         for p in (g0, be0, g1, be1, ge0, bee0, gx0, bxe0)], axis=1)

    shared = {"w0": w0, "we1": we1, "wsm": wsm, "wx1": wx1,
              "brow": brow, "lnp": lnp}
    in_maps = []
    for c in range(NCORES):
        m = dict(shared)
        m["adjT"] = adjT[c * TPC:(c + 1) * TPC]
        in_maps.append(m)
    # output bias row (edge-head bias + exit bias), applied on the host
    obias = np.concatenate(
        [np.asarray(beb1, np.float32).ravel(),
         np.asarray(bx1, np.float32).ravel()])
    return in_maps, obias


def kernel(**inputs) -> np.ndarray:
    global LAST_EXEC_NS
    if "nc" not in _cache:
        _cache["nc"] = _build()
    nc = _cache["nc"]

    in_maps, obias = _prep(**inputs)
    trace = bool(int(os.environ.get("KERNEL_TRACE", "0")))
    kw = {}
    if trace:
        base = os.environ.get("KERNEL_TRACE_DIR") or None
        td = None
        if base:
            import tempfile
            os.makedirs(base, exist_ok=True)
            td = tempfile.mkdtemp(dir=base)
        kw = dict(trace=True, tmpdir=td)
    res = run_bass_kernel_spmd(nc, in_maps, core_ids=list(range(NCORES)), **kw)
    LAST_EXEC_NS = res.exec_time_ns
    out = np.concatenate(
        [np.asarray(res.results[c]["out"]) for c in range(NCORES)], axis=0)
    return out.astype(np.float32) + obias[None, :]


# revision 36
# speedup vs baseline: 1.0706x; 1.0706x over previous
"""Trainium2 Bass kernel for the AdjacencyPolicyModule MLP.

Strategy (data-parallel over batch, 8 NeuronCores, 512 graphs/core):
  - Host: scatter edge lists into a dense transposed adjacency laid out as
    [tile, dst, src*128+g] in fp8e4 (values 0/1 are exact; bytes written
    directly, no float conversion). Weights converted to bf16 and
    pre-arranged into the chunked SBUF layouts the matmuls want.
  - Device pass 1, software-pipelined over 128-graph tiles:
      step t:   emb0(t)  = adjT(t) x W0   (128 fp8xbf16 matmuls + bias row)
      step t+1: epiA(t)  = LN+ReLU -> h0T transposes -> W1 matmuls
      step t+2: epiB(t)  = LN+ReLU -> h1T transposes -> Wm matmuls -> stash
    so the PE chews tile t+1's big matmul group while tile t's LayerNorm
    chain runs on DVE/ACT.
  - Device pass 2, same 2-stage pipeline:
      A(t): embT transposes, We0/Wx0 matmuls (launches LN chains)
      B(t): heT/hxT transposes, exit head, 32 edge-head column matmuls,
            stage copies, output DMA in 4 column groups.
  - Output staged in bf16; host upcasts to f32 and concatenates.
"""

import os

import numpy as np
import ml_dtypes

import concourse.bass as bass
import concourse.tile as tile
from concourse import bacc, mybir
from concourse.bass_utils import run_bass_kernel_spmd
from concourse.masks import make_identity
from concourse.tile_rust import add_dep_helper

B, N, D = 4096, 128, 256
EO = N * N - N          # 16256 edge-action outputs
OUTW = EO + 1           # 16257 with exit action
NCORES = 8
GPC = B // NCORES       # 512 graphs per core
TPC = GPC // 128        # 4 row tiles per core
COLT = 508              # edge-head column tile (32 * 508 = 16256)
NCOL = EO // COLT

BF = ml_dtypes.bfloat16
F8 = ml_dtypes.float8_e4m3

_cache = {}

LAST_EXEC_NS = None


class Ctx:
    """Shared handles for the kernel builder."""
    pass


def _ln_relu(nc, cx, x_ps, gi, bi, out_bf):
    """out_bf = relu(layernorm(x_ps) * g + be), f32 PSUM in, bf16 SBUF out."""
    acts = cx.acts
    stats = acts.tile([128, 6], mybir.dt.float32, tag="stats", bufs=2)
    nc.vector.bn_stats(out=stats[:], in_=x_ps[:])
    mv = acts.tile([128, 2], mybir.dt.float32, tag="mv", bufs=2)
    nc.vector.bn_aggr(out=mv[:], in_=stats[:])
    rstd = acts.tile([128, 1], mybir.dt.float32, tag="rstd", bufs=2)
    nc.scalar.activation(
        out=rstd[:], in_=mv[:, 1:2],
        func=mybir.ActivationFunctionType.Sqrt,
        bias=cx.eps[:, 0:1], scale=1.0,
    )
    nc.vector.reciprocal(out=rstd[:], in_=rstd[:])
    xn = acts.tile([128, D], mybir.dt.float32, tag="xn", bufs=3)
    nc.vector.tensor_scalar(
        out=xn[:], in0=x_ps[:],
        scalar1=mv[:, 0:1], scalar2=rstd[:, 0:1],
        op0=mybir.AluOpType.subtract, op1=mybir.AluOpType.mult,
    )
    nc.vector.tensor_tensor(
        out=xn[:], in0=xn[:], in1=cx.lnp[:, gi * D:(gi + 1) * D],
        op=mybir.AluOpType.mult,
    )
    nc.vector.tensor_tensor(
        out=xn[:], in0=xn[:], in1=cx.lnp[:, bi * D:(bi + 1) * D],
        op=mybir.AluOpType.add,
    )
    nc.scalar.activation(
        out=out_bf[:], in_=xn[:], func=mybir.ActivationFunctionType.Relu,
    )


def _transpose(nc, cx, src_bf):
    """src_bf [128(g), 256(d)] bf16 -> [128(d%128), k*128+g] bf16 (2 chunks)."""
    dst = cx.acts.tile([128, D], mybir.dt.bfloat16, tag="hT", bufs=3)
    for k in range(2):
        pt = cx.psT.tile([128, 128], mybir.dt.bfloat16, tag="tr")
        nc.tensor.transpose(
            out=pt[:], in_=src_bf[:, k * 128:(k + 1) * 128], identity=cx.ident[:],
        )
        nc.vector.tensor_copy(out=dst[:, k * 128:(k + 1) * 128], in_=pt[:])
    return dst


def _dense_small(nc, cx, pool, tag, lhsT, widx, boff):
    """[128,256] = lhsT.T @ W_widx + bias; returns PSUM tile."""
    ps = pool.tile([128, D], mybir.dt.float32, tag=tag)
    for k in range(2):
        nc.tensor.matmul(
            ps[:], lhsT=lhsT[:, k * 128:(k + 1) * 128],
            rhs=cx.wsm[:, widx * 2 * D + k * D: widx * 2 * D + (k + 1) * D],
            start=(k == 0), stop=False,
        )
    nc.tensor.matmul(
        ps[:], lhsT=cx.ones[0:1, :], rhs=cx.brow[0:1, boff:boff + D],
        start=False, stop=True,
    )
    return ps


def _build():
    nc = bacc.Bacc("TRN2", target_bir_lowering=False, debug=False,
                   num_devices=NCORES)
    dt = mybir.dt

    adjT_d = nc.declare_dram_parameter("adjT", [TPC, 128, N * 128], dt.float8e4, isOutput=False)
    w0_d = nc.declare_dram_parameter("w0", [128, 128 * D], dt.bfloat16, isOutput=False)
    we1_d = nc.declare_dram_parameter("we1", [128, 2 * EO], dt.bfloat16, isOutput=False)
    wsm_d = nc.declare_dram_parameter("wsm", [4, 128, 2 * D], dt.bfloat16, isOutput=False)
    wx1_d = nc.declare_dram_parameter("wx1", [128, 2], dt.bfloat16, isOutput=False)
    brow_d = nc.declare_dram_parameter("brow", [1, 5 * D + 1], dt.bfloat16, isOutput=False)
    lnp_d = nc.declare_dram_parameter("lnp", [128, 8 * D], dt.float32, isOutput=False)
    out_d = nc.declare_dram_parameter("out", [GPC, OUTW], dt.bfloat16, isOutput=True)

    cx = Ctx()

    with tile.TileContext(nc) as tc:
        with (
            tc.tile_pool(name="persist", bufs=1) as persist,
            tc.tile_pool(name="acts", bufs=3) as acts,
        ):
            cx.acts = acts

            # ---- pass 1 (with critical-path-first DMA ordering) ----
            with tc.tile_pool(name="w0p", bufs=1) as w0p, \
                 tc.tile_pool(name="adjp", bufs=8) as adjp, \
                 tc.tile_pool(name="psE", bufs=2, space="PSUM") as psE, \
                 tc.tile_pool(name="psH", bufs=2, space="PSUM") as psH, \
                 tc.tile_pool(name="psT", bufs=2, space="PSUM") as psT, \
                 tc.tile_pool(name="psM", bufs=2, space="PSUM") as psM:
                cx.psT = psT

                # Quartered adjacency chunks for tile 0 interleaved with W0
                # slices in exact matmul consumption order, all on the sync
                # HWDGE ring (FIFO), so the PE starts ~4us in and is paced by
                # DMA with minimal stalls.
                QW = 32 * 128  # quarter width (fp8 bytes)

                def adj_quarter(t, q):
                    ah = adjp.tile([128, QW], dt.float8e4, tag="adj")
                    nc.sync.dma_start(
                        out=ah[:], in_=adjT_d[t, :, q * QW:(q + 1) * QW])
                    return ah

                adj_q = []
                w0t = [None] * 8
                for q in range(4):
                    adj_q.append(adj_quarter(0, q))
                    for i in (2 * q, 2 * q + 1):
                        w = w0p.tile([128, 16 * D], dt.bfloat16, tag=f"w0_{i}")
                        nc.sync.dma_start(
                            out=w[:], in_=w0_d[:, i * 16 * D:(i + 1) * 16 * D])
                        w0t[i] = w
                for q in range(4):
                    adj_q.append(adj_quarter(1, q))

                # small persistent constants: also on the sync ring so the
                # FIFO delays them behind the critical tile-0/1 loads
                wsm = persist.tile([128, 4 * 2 * D], dt.bfloat16)
                for i in range(4):
                    nc.sync.dma_start(
                        out=wsm[:, i * 2 * D:(i + 1) * 2 * D], in_=wsm_d[i])
                wx1 = persist.tile([128, 2], dt.bfloat16)
                nc.sync.dma_start(out=wx1[:], in_=wx1_d[:])
                brow = persist.tile([1, 5 * D + 1], dt.bfloat16)
                nc.sync.dma_start(out=brow[:], in_=brow_d[:])
                lnp = persist.tile([128, 8 * D], dt.float32)
                nc.sync.dma_start(out=lnp[:], in_=lnp_d[:])
                ident = persist.tile([128, 128], dt.bfloat16)
                make_identity(nc, ident[:])
                ones = persist.tile([1, 128], dt.bfloat16)
                nc.vector.memset(ones[:], 1.0)
                eps = persist.tile([128, 1], dt.float32)
                nc.vector.memset(eps[:], 1e-5)
                emb_stash = persist.tile([128, TPC * D], dt.bfloat16)
                we1 = persist.tile([128, 2 * EO], dt.bfloat16)

                cx.wsm, cx.wx1, cx.brow, cx.lnp = wsm, wx1, brow, lnp
                cx.ident, cx.ones, cx.eps = ident, ones, eps
                cx.emb_stash, cx.we1 = emb_stash, we1

                # pipeline state
                emb0_ps = [None] * TPC
                emb0_last = [None] * TPC
                h0_bf = [None] * TPC
                h1_ps = [None] * TPC
                h1_bf = [None] * TPC

                def stage_emb0(t):
                    adj = adj_q[:4]
                    del adj_q[:4]
                    ps = psE.tile([128, D], dt.float32, tag="e0")
                    for s in range(128):
                        nc.tensor.matmul(
                            ps[:],
                            lhsT=adj[s // 32][:, (s % 32) * 128:(s % 32 + 1) * 128],
                            rhs=w0t[s // 16][:, (s % 16) * D:(s % 16 + 1) * D],
                            start=(s == 0), stop=False,
                        )
                    emb0_last[t] = nc.tensor.matmul(
                        ps[:], lhsT=ones[0:1, :], rhs=brow[0:1, 0:D],
                        start=False, stop=True,
                    )
                    emb0_ps[t] = ps
                    # prefetch the next-next tile's adjacency
                    if t + 2 < TPC:
                        for q in range(4):
                            adj_q.append(adj_quarter(t + 2, q))

                def stage_epiA(t):
                    h0 = acts.tile([128, D], dt.bfloat16, tag="h", bufs=4)
                    _ln_relu(nc, cx, emb0_ps[t], 0, 1, h0)
                    h0_bf[t] = h0
                    h0T = _transpose(nc, cx, h0)
                    h1_ps[t] = _dense_small(nc, cx, psH, "h1", h0T, 0, D)

                def stage_epiB(t):
                    h1 = acts.tile([128, D], dt.bfloat16, tag="h", bufs=4)
                    _ln_relu(nc, cx, h1_ps[t], 2, 3, h1)
                    h1_bf[t] = h1
                    h1T = _transpose(nc, cx, h1)
                    # embT computed directly: embT[d2,g] = sum_d1 Wm[d1,d2]
                    # h1T[d1,g] + bm[d2]; avoids a later transpose on the
                    # A-stage critical path.
                    for h in range(2):
                        eps_h = psM.tile([128, 128], dt.float32, tag="em")
                        for k in range(2):
                            nc.tensor.matmul(
                                eps_h[:],
                                lhsT=wsm[:, 2 * D + k * D + h * 128:
                                         2 * D + k * D + (h + 1) * 128],
                                rhs=h1T[:, k * 128:(k + 1) * 128],
                                start=(k == 0), stop=False)
                        nc.tensor.matmul(
                            eps_h[:],
                            lhsT=brow[0:1, 2 * D + h * 128:2 * D + (h + 1) * 128],
                            rhs=ones[0:1, :], start=False, stop=True)
                        nc.vector.tensor_copy(
                            out=emb_stash[:, t * D + h * 128:t * D + (h + 1) * 128],
                            in_=eps_h[:])

                heT_t = [None] * TPC
                hxT_t = [None] * TPC

                def stage_A(t, pool, tag):
                    embT = emb_stash[:, t * D:(t + 1) * D]
                    he_ps = _dense_small(nc, cx, pool, tag, embT, 2, 3 * D)
                    hx_ps = _dense_small(nc, cx, pool, tag, embT, 3, 4 * D)
                    he = acts.tile([128, D], dt.bfloat16, tag="ho", bufs=6)
                    _ln_relu(nc, cx, he_ps, 4, 5, he)
                    hx = acts.tile([128, D], dt.bfloat16, tag="ho", bufs=6)
                    _ln_relu(nc, cx, hx_ps, 6, 7, hx)
                    heT_t[t] = he
                    hxT_t[t] = hx

                for step in range(TPC + 2):
                    if step < TPC:
                        stage_emb0(step)
                    if 1 <= step < TPC + 1:
                        stage_epiA(step - 1)
                    if step >= 2:
                        stage_epiB(step - 2)
                    # head LN chains for tiles 0/1 emitted in the pass-1 tail
                    if step in (4, 5):
                        stage_A(step - 4, psM, "em")

                # we1 trickles in behind the critical path: each slice DMA is
                # anchored to a layer-0 matmul group so it does not compete
                # with adjT/w0 for HBM bandwidth at startup.
                W14 = EO // 2  # 8128 = 16 col groups per slice
                anchors = [1, 1, 2, 2]
                for i in range(4):
                    dma = nc.gpsimd.dma_start(
                        out=we1[:, i * W14:(i + 1) * W14],
                        in_=we1_d[:, i * W14:(i + 1) * W14])
                    add_dep_helper(
                        dma.ins, emb0_last[anchors[i]].ins,
                        reason="delay we1 load behind layer-0")

            # ---- pass 2: heads ----
            with tc.tile_pool(name="stage", bufs=2) as stagep, \
                 tc.tile_pool(name="psHead", bufs=2, space="PSUM") as psHead, \
                 tc.tile_pool(name="psT2", bufs=2, space="PSUM") as psT2, \
                 tc.tile_pool(name="psBig", bufs=4, space="PSUM") as psBig:
                cx.psT = psT2

                def stage_B(t):
                    heT = _transpose(nc, cx, heT_t[t])
                    hxT = _transpose(nc, cx, hxT_t[t])
                    stage = stagep.tile([128, OUTW], dt.bfloat16)

                    # exit head (bias bx1 applied host-side)
                    exps = psBig.tile([128, COLT], dt.float32, tag="big")
                    for k in range(2):
                        nc.tensor.matmul(
                            exps[:, 0:1], lhsT=hxT[:, k * 128:(k + 1) * 128],
                            rhs=wx1[:, k:k + 1], start=(k == 0), stop=(k == 1))
                    nc.vector.tensor_copy(
                        out=stage[:, EO:EO + 1], in_=exps[:, 0:1])

                    # edge head (bias beb1 applied host-side); we1 is stored
                    # k-interleaved per column group. Output DMA'd in column
                    # groups, with a small final group to shorten the tail.
                    dma_at = {7: (0, 8), 15: (8, 16), 23: (16, 24),
                              29: (24, 30), 31: (30, 32)}
                    for c in range(NCOL):
                        pb = psBig.tile([128, COLT], dt.float32, tag="big")
                        for k in range(2):
                            nc.tensor.matmul(
                                pb[:],
                                lhsT=heT[:, k * 128:(k + 1) * 128],
                                rhs=we1[:, (2 * c + k) * COLT:(2 * c + k + 1) * COLT],
                                start=(k == 0), stop=(k == 1))
                        if c % 2 == 0:
                            nc.vector.tensor_copy(
                                out=stage[:, c * COLT:(c + 1) * COLT], in_=pb[:])
                        else:
                            nc.scalar.activation(
                                out=stage[:, c * COLT:(c + 1) * COLT], in_=pb[:],
                                func=mybir.ActivationFunctionType.Copy)
                        if c in dma_at:
                            glo, ghi = dma_at[c]
                            lo = glo * COLT
                            hi = ghi * COLT if ghi < NCOL else OUTW
                            nc.sync.dma_start(
                                out=out_d[t * 128:(t + 1) * 128, lo:hi],
                                in_=stage[:, lo:hi])

                sched = [("B", 0), ("A", 2), ("B", 1), ("A", 3), ("B", 2),
                         ("B", 3)]
                for kind, t in sched:
                    if kind == "A":
                        stage_A(t, psHead, "head")
                    else:
                        stage_B(t)

    nc.compile()
    return nc


def _prep(edge_src, edge_dst, edge_batch,
          W0, b0, g0, be0, W1, b1, g1, be1, Wm, bm,
          We0, beb0, ge0, bee0, We1, beb1,
          Wx0, bx0, gx0, bxe0, Wx1, bx1):
    # adjacency: [32 tiles, dst, src, g], fp8 bytes written directly
    A = np.zeros((B, N, N), np.uint8)
    A[np.asarray(edge_batch), np.asarray(edge_src), np.asarray(edge_dst)] = 0x38
    adjT = np.ascontiguousarray(
        A.reshape(B // 128, 128, N, N).transpose(0, 3, 2, 1)
    ).reshape(B // 128, 128, N * 128).view(F8)

    def chunked(W, dout):
        # [K, dout] -> [128, (K//128)*dout] with [p, k*dout+j] = W[k*128+p, j]
        K = W.shape[0]
        return np.ascontiguousarray(
            np.asarray(W, np.float32).reshape(K // 128, 128, dout)
            .transpose(1, 0, 2)).astype(BF).reshape(128, (K // 128) * dout)

    w0 = chunked(W0, D)
    # We1 k-interleaved per column group: [p, (c*2+k)*COLT+j] = We1[k*128+p, c*COLT+j]
    we1 = np.ascontiguousarray(
        np.asarray(We1, np.float32).reshape(2, 128, NCOL, COLT)
        .transpose(1, 2, 0, 3)).astype(BF).reshape(128, 2 * EO)
    wsm = np.stack([chunked(W, D) for W in (W1, Wm, We0, Wx0)])
    wx1 = chunked(Wx1, 1)
    brow = np.concatenate(
        [np.asarray(v, np.float32).ravel() for v in (b0, b1, bm, beb0, bx0, bx1)]
    )[None, :].astype(BF)
    lnp = np.concatenate(
        [np.tile(np.asarray(p, np.float32)[None, :], (128, 1))
         for p in (g0, be0, g1, be1, ge0, bee0, gx0, bxe0)], axis=1)

    shared = {"w0": w0, "we1": we1, "wsm": wsm, "wx1": wx1,
              "brow": brow, "lnp": lnp}
    in_maps = []
    for c in range(NCORES):
        m = dict(shared)
        m["adjT"] = adjT[c * TPC:(c + 1) * TPC]
        in_maps.append(m)
    # output bias row (edge-head bias + exit bias), applied on the host
    obias = np.concatenate(
        [np.asarray(beb1, np.float32).ravel(),
         np.asarray(bx1, np.float32).ravel()])
    return in_maps, obias


def kernel(**inputs) -> np.ndarray:
    global LAST_EXEC_NS
    if "nc" not in _cache:
        _cache["nc"] = _build()
    nc = _cache["nc"]

    in_maps, obias = _prep(**inputs)
    trace = bool(int(os.environ.get("KERNEL_TRACE", "0")))
    kw = {}
    if trace:
        base = os.environ.get("KERNEL_TRACE_DIR") or None
        td = None
        if base:
            import tempfile
            os.makedirs(base, exist_ok=True)
            td = tempfile.mkdtemp(dir=base)
        kw = dict(trace=True, tmpdir=td)
    res = run_bass_kernel_spmd(nc, in_maps, core_ids=list(range(NCORES)), **kw)
    LAST_EXEC_NS = res.exec_time_ns
    out = np.concatenate(
        [np.asarray(res.results[c]["out"]) for c in range(NCORES)], axis=0)
    return out.astype(np.float32) + obias[None, :]
